# revision 10
# baseline (speedup 1.0000x reference)
"""BinarizeConv2dSDP kernel for Trainium2 (8 NeuronCores, data-parallel over batch).

out = conv2d(sign(x), sign(M + sum_k rv[k] * Z[k]), stride 1, pad 1) * Alpha

Key simplification: the reference normalizes (M, Z) by rsqrt(M^2 + sum Z^2 / SCALE)
before forming w = rv@Z + M, but that factor is strictly positive and applied
multiplicatively to the whole expression, so sign(w) is unaffected.  The binary
weights are just sign(M + sum_k rv[k] Z[k]).

Strategy per core (8 images each):
  - weight gen on DVE: w = M + sum rv_k Z_k in [oc, ic*9] layout, sign -> bf16,
    then 9 PE transposes give lhsT tiles [ic, oc] per kernel tap.
  - conv as 9 accumulated matmuls per output chunk: contraction over ic=128
    partitions, PSUM accumulates the 3x3 taps over a 58-wide zero-padded
    activation image (sign(x) in bf16).
  - Alpha scaling on DVE during PSUM->SBUF eviction, one output DMA per image.
"""

import numpy as np
from contextlib import ExitStack

import concourse.bass as bass
import concourse.mybir as mybir
import concourse.tile as tile
from concourse.bacc import Bacc
from concourse.bass_utils import run_bass_kernel_spmd

N_CORES = 8
B, C, H, W = 64, 128, 56, 56
BPC = B // N_CORES  # images per core
KS, K = 3, 5
PH, PW = H + 2, W + 2  # zero-padded image
CHUNK_ROWS = 8
N_CHUNKS = H // CHUNK_ROWS
FREE = CHUNK_ROWS * W  # matmul free dim (448 <= 512 psum bank)
F32 = mybir.dt.float32
BF16 = mybir.dt.bfloat16


def build_kernel(rv_vals):
    """Build the single-core Bass module (SPMD: same program on all 8 cores).

    rv_vals: the 5 rv scalars, baked as immediates into the weight-gen ops.
    """
    # Bacc (not plain Bass): its compile() pass pipeline legalizes sync waits
    # (TRN2 allows at most 1 embedded wait per engine instruction; excess waits
    # are split into InstEventSemaphore via generate_event_semaphores).
    nc = Bacc()
    x_p = nc.declare_dram_parameter("x", [BPC, C, H, W], F32, isOutput=False)
    m_p = nc.declare_dram_parameter("M", [C, C, KS, KS], F32, isOutput=False)
    z_p = nc.declare_dram_parameter("Z", [K, C, C, KS, KS], F32, isOutput=False)
    a_p = nc.declare_dram_parameter("Alpha", [C, 1, 1], F32, isOutput=False)
    rv_p = nc.declare_dram_parameter("rv", [1, K], F32, isOutput=False)
    out_p = nc.declare_dram_parameter("out", [BPC, C, H, W], F32, isOutput=True)

    from concourse.masks import make_identity

    NW = C * KS * KS  # 1152 weight elements per out-channel row

    with tile.TileContext(nc) as tc, ExitStack() as ctx:
        const = ctx.enter_context(tc.tile_pool(name="const", bufs=1))
        wg = ctx.enter_context(tc.tile_pool(name="wg", bufs=1))
        zpool = ctx.enter_context(tc.tile_pool(name="zpool", bufs=1))
        xin = ctx.enter_context(tc.tile_pool(name="xin", bufs=3))
        pad = ctx.enter_context(tc.tile_pool(name="pad", bufs=2))
        opool = ctx.enter_context(tc.tile_pool(name="opool", bufs=2))
        ps_t = ctx.enter_context(tc.tile_pool(name="ps_t", bufs=2, space="PSUM"))
        ps_c = ctx.enter_context(tc.tile_pool(name="ps_c", bufs=4, space="PSUM"))

        # ---- constants ----
        identity = const.tile([C, C], BF16)
        make_identity(nc, identity)
        alpha_sb = const.tile([C, 1], F32)
        nc.sync.dma_start(alpha_sb[:], a_p[:].rearrange("c a b -> c (a b)"))
        # rv is consumed via baked immediates; still touch the input so the
        # parameter is used (run path requires binding it anyway).
        rv_sb = const.tile([1, K], F32)
        nc.sync.dma_start(rv_sb[:], rv_p[:])

        # ---- weight generation: w = M + sum_k rv_k Z_k, in [oc, ic*3*3] ----
        m_sb = wg.tile([C, NW], F32)
        nc.sync.dma_start(m_sb[:], m_p[:].rearrange("o i a b -> o (i a b)"))
        z_sbs = []
        for k in range(K):
            z_sb = zpool.tile([C, NW], F32, tag=f"z{k}")
            nc.sync.dma_start(z_sb[:], z_p[k].rearrange("o i a b -> o (i a b)"))
            z_sbs.append(z_sb)
        # Absorb the DMA-completion waits with cheap copies: the TensorScalarPtr
        # (S2S2D2_STT) instruction format below cannot carry sync waits, so its
        # DVE dependencies must already be settled in the engine's vector clock.
        scratch = wg.tile([C, 8], F32)
        for k in range(K):
            nc.vector.tensor_copy(scratch[:, k : k + 1], z_sbs[k][:, 0:1])
        nc.vector.tensor_copy(scratch[0:1, K : K + 1], rv_sb[0:1, 0:1])
        nc.vector.tensor_copy(scratch[:, K + 1 : K + 2], m_sb[:, 0:1])
        w_sb = wg.tile([C, NW], F32)
        nc.vector.tensor_copy(w_sb[:], m_sb[:])
        for k in range(K):
            # w = (z_k * rv_k) + w
            nc.vector.scalar_tensor_tensor(
                w_sb[:],
                z_sbs[k][:],
                float(rv_vals[k]),
                w_sb[:],
                mybir.AluOpType.mult,
                mybir.AluOpType.add,
            )
        bw_sb = wg.tile([C, NW], BF16)
        nc.scalar.sign(bw_sb[:], w_sb[:])

        # transpose each tap's [oc, ic] into lhsT [ic, oc]
        wt = const.tile([C, KS * KS, C], BF16)
        bw_r = bw_sb[:].rearrange("o (i j) -> o i j", j=KS * KS)
        for j in range(KS * KS):
            tp = ps_t.tile([C, C], BF16)
            nc.tensor.transpose(tp[:], bw_r[:, :, j], identity[:])
            nc.vector.tensor_copy(wt[:, j, :], tp[:])

        # ---- conv main loop ----
        x_ap = x_p[:]
        o_ap = out_p[:]
        taps = [(ky, kx) for ky in range(KS) for kx in range(KS)]
        for i in range(BPC):
            x_sb = xin.tile([C, H * W], F32)
            nc.sync.dma_start(x_sb[:], x_ap[i].rearrange("c h w -> c (h w)"))
            ba = pad.tile([C, PH * PW], BF16)
            nc.gpsimd.memset(ba[:], 0.0)
            ba_r = ba[:].rearrange("c (h w) -> c h w", w=PW)
            nc.scalar.sign(
                ba_r[:, 1 : H + 1, 1 : W + 1],
                x_sb[:].rearrange("c (h w) -> c h w", w=W),
            )
            o_sb = opool.tile([C, H * W], F32)
            for ch in range(N_CHUNKS):
                pt = ps_c.tile([C, FREE], F32)
                r0 = ch * CHUNK_ROWS
                for j, (ky, kx) in enumerate(taps):
                    nc.tensor.matmul(
                        pt[:],
                        wt[:, j, :],
                        ba_r[:, r0 + ky : r0 + ky + CHUNK_ROWS, kx : kx + W],
                        start=(j == 0),
                        stop=(j == KS * KS - 1),
                    )
                # PSUM -> SBUF eviction with per-channel Alpha scale on ScalarE
                # (ACTIVATE carries sync waits fine; DVE TensorScalarPtr cannot)
                nc.scalar.mul(
                    o_sb[:, ch * FREE : (ch + 1) * FREE], pt[:], alpha_sb[:, 0:1]
                )
            nc.sync.dma_start(o_ap[i].rearrange("c h w -> c (h w)"), o_sb[:])

    nc.finalize()
    return nc


_CACHE = {}


def _get_nc(rv):
    key = rv.tobytes()
    if key not in _CACHE:
        _CACHE[key] = build_kernel(np.asarray(rv, np.float32).reshape(-1))
    return _CACHE[key]


def _run(inputs, trace=False):
    x = np.ascontiguousarray(np.asarray(inputs["x"], np.float32))
    M = np.ascontiguousarray(np.asarray(inputs["M"], np.float32))
    Z = np.ascontiguousarray(np.asarray(inputs["Z"], np.float32))
    Alpha = np.ascontiguousarray(np.asarray(inputs["Alpha"], np.float32))
    rv = np.ascontiguousarray(np.asarray(inputs["rv"], np.float32))
    nc = _get_nc(rv)
    in_maps = [
        {"x": x[c * BPC : (c + 1) * BPC], "M": M, "Z": Z, "Alpha": Alpha, "rv": rv}
        for c in range(N_CORES)
    ]
    res = run_bass_kernel_spmd(nc, in_maps, list(range(N_CORES)), trace=trace)
    out = np.concatenate([res.results[c]["out"] for c in range(N_CORES)], axis=0)
    return out, res


def kernel(**inputs):
    out, _ = _run(inputs, trace=False)
    return out


def kernel_traced(**inputs):
    out, res = _run(inputs, trace=True)
    return out, res


# revision 15
# speedup vs baseline: 1.1219x; 1.1219x over previous
"""BinarizeConv2dSDP kernel for Trainium2 (8 NeuronCores, data-parallel over batch).

out = conv2d(sign(x), sign(M + sum_k rv[k] * Z[k]), stride 1, pad 1) * Alpha

Key simplification: the reference normalizes (M, Z) by rsqrt(M^2 + sum Z^2 / SCALE)
before forming w = rv@Z + M, but that factor is strictly positive and applied
multiplicatively to the whole expression, so sign(w) is unaffected.  The binary
weights are just sign(M + sum_k rv[k] Z[k]).

Strategy per core (8 images each):
  - weight gen on DVE: w = M + sum rv_k Z_k in [oc, ic*9] layout, sign -> bf16,
    then 9 PE transposes give lhsT tiles [ic, oc] per kernel tap.
  - conv as 9 accumulated matmuls per output chunk: contraction over ic=128
    partitions, PSUM accumulates the 3x3 taps over a 58-wide zero-padded
    activation image (sign(x) in bf16).
  - Alpha scaling on DVE during PSUM->SBUF eviction, one output DMA per image.
"""

import numpy as np
from contextlib import ExitStack

import concourse.bass as bass
import concourse.mybir as mybir
import concourse.tile as tile
from concourse.bacc import Bacc
from concourse.bass_utils import run_bass_kernel_spmd

N_CORES = 8
B, C, H, W = 64, 128, 56, 56
BPC = B // N_CORES  # images per core
KS, K = 3, 5
PH, PW = H + 2, W + 2  # zero-padded image
CHUNK_ROWS = 8
N_CHUNKS = H // CHUNK_ROWS
FREE = CHUNK_ROWS * W  # matmul free dim (448 <= 512 psum bank)
F32 = mybir.dt.float32
BF16 = mybir.dt.bfloat16
F8 = mybir.dt.float8e4


def build_kernel(rv_vals):
    """Build the single-core Bass module (SPMD: same program on all 8 cores).

    rv_vals: the 5 rv scalars, baked as immediates into the weight-gen ops.
    """
    # Bacc (not plain Bass): its compile() pass pipeline legalizes sync waits
    # (TRN2 allows at most 1 embedded wait per engine instruction; excess waits
    # are split into InstEventSemaphore via generate_event_semaphores).
    nc = Bacc()
    x_p = nc.declare_dram_parameter("x", [BPC, C, H, W], F32, isOutput=False)
    m_p = nc.declare_dram_parameter("M", [C, C, KS, KS], F32, isOutput=False)
    z_p = nc.declare_dram_parameter("Z", [K, C, C, KS, KS], F32, isOutput=False)
    a_p = nc.declare_dram_parameter("Alpha", [C, 1, 1], F32, isOutput=False)
    rv_p = nc.declare_dram_parameter("rv", [1, K], F32, isOutput=False)
    out_p = nc.declare_dram_parameter("out", [BPC, C, H, W], F32, isOutput=True)

    from concourse.masks import make_identity

    NW = C * KS * KS  # 1152 weight elements per out-channel row

    with tile.TileContext(nc) as tc, ExitStack() as ctx:
        const = ctx.enter_context(tc.tile_pool(name="const", bufs=1))
        wg = ctx.enter_context(tc.tile_pool(name="wg", bufs=1))
        zpool = ctx.enter_context(tc.tile_pool(name="zpool", bufs=1))
        xin = ctx.enter_context(tc.tile_pool(name="xin", bufs=3))
        pad = ctx.enter_context(tc.tile_pool(name="pad", bufs=2))
        opool = ctx.enter_context(tc.tile_pool(name="opool", bufs=2))
        ps_t = ctx.enter_context(tc.tile_pool(name="ps_t", bufs=2, space="PSUM"))
        ps_c = ctx.enter_context(tc.tile_pool(name="ps_c", bufs=4, space="PSUM"))

        # ---- constants ----
        identity = const.tile([C, C], BF16)
        make_identity(nc, identity)
        alpha_sb = const.tile([C, 1], F32)
        nc.sync.dma_start(alpha_sb[:], a_p[:].rearrange("c a b -> c (a b)"))
        # rv is consumed via baked immediates; still touch the input so the
        # parameter is used (run path requires binding it anyway).
        rv_sb = const.tile([1, K], F32)
        nc.sync.dma_start(rv_sb[:], rv_p[:])

        # ---- weight generation: w = M + sum_k rv_k Z_k, in [oc, ic*3*3] ----
        m_sb = wg.tile([C, NW], F32)
        nc.sync.dma_start(m_sb[:], m_p[:].rearrange("o i a b -> o (i a b)"))
        z_sbs = []
        for k in range(K):
            z_sb = zpool.tile([C, NW], F32, tag=f"z{k}")
            nc.sync.dma_start(z_sb[:], z_p[k].rearrange("o i a b -> o (i a b)"))
            z_sbs.append(z_sb)
        # Absorb the DMA-completion waits with cheap copies: the TensorScalarPtr
        # (S2S2D2_STT) instruction format below cannot carry sync waits, so its
        # DVE dependencies must already be settled in the engine's vector clock.
        scratch = wg.tile([C, 8], F32)
        for k in range(K):
            nc.vector.tensor_copy(scratch[:, k : k + 1], z_sbs[k][:, 0:1])
        nc.vector.tensor_copy(scratch[0:1, K : K + 1], rv_sb[0:1, 0:1])
        nc.vector.tensor_copy(scratch[:, K + 1 : K + 2], m_sb[:, 0:1])
        w_sb = wg.tile([C, NW], F32)
        nc.vector.tensor_copy(w_sb[:], m_sb[:])
        for k in range(K):
            # w = (z_k * rv_k) + w
            nc.vector.scalar_tensor_tensor(
                w_sb[:],
                z_sbs[k][:],
                float(rv_vals[k]),
                w_sb[:],
                mybir.AluOpType.mult,
                mybir.AluOpType.add,
            )
        bw_sb = wg.tile([C, NW], BF16)
        nc.scalar.sign(bw_sb[:], w_sb[:])

        # transpose each tap's [oc, ic] into lhsT [ic, oc]; store as fp8e4
        # (+-1 is exact) with a 10th all-zero tap so the 9 taps pair up into
        # 5 DoubleRow matmuls (2 fp8 weights per PE cell = 2x throughput).
        wt = const.tile([C, KS * KS + 1, C], F8)
        nc.vector.memset(wt[:, KS * KS, :], 0.0)
        bw_r = bw_sb[:].rearrange("o (i j) -> o i j", j=KS * KS)
        for j in range(KS * KS):
            tp = ps_t.tile([C, C], BF16)
            nc.tensor.transpose(tp[:], bw_r[:, :, j], identity[:])
            nc.vector.tensor_copy(wt[:, j, :], tp[:])

        # ---- conv main loop ----
        x_ap = x_p[:]
        o_ap = out_p[:]

        def tap_off(r0, j):
            # flat offset of (out-row r0, tap j)'s top-left read in the padded image
            if j == KS * KS:  # zero tap: alias tap 8's window (weights are 0)
                j = KS * KS - 1
            return (r0 + j // KS) * PW + (j % KS)

        # Matmul free dim spans whole padded rows (8*58=464 contiguous, so the
        # DoubleRow moving AP stays 3D [C, 2, 464]); each row's last 2 output
        # columns are garbage that the eviction skips.  +2 trailing elements
        # keep the last tap's 464-wide read in bounds.
        FREE_R = CHUNK_ROWS * PW  # 464 <= 512 psum bank
        for i in range(BPC):
            x_sb = xin.tile([C, H * W], F32)
            nc.sync.dma_start(x_sb[:], x_ap[i].rearrange("c h w -> c (h w)"))
            ba = pad.tile([C, PH * PW + 2], F8)
            nc.gpsimd.memset(ba[:], 0.0)
            ba_r = ba[:, 0 : PH * PW].rearrange("c (h w) -> c h w", w=PW)
            nc.scalar.sign(
                ba_r[:, 1 : H + 1, 1 : W + 1],
                x_sb[:].rearrange("c (h w) -> c h w", w=W),
            )
            o_sb = opool.tile([C, H * W], F32)
            for ch in range(N_CHUNKS):
                pt = ps_c.tile([C, FREE_R], F32)
                r0 = ch * CHUNK_ROWS
                for p in range(5):
                    o0 = tap_off(r0, 2 * p)
                    o1 = tap_off(r0, 2 * p + 1)
                    rhs = bass.AP(
                        ba[:].tensor,
                        o0,
                        [[PH * PW + 2, C], [o1 - o0, 2], [1, FREE_R]],
                    )
                    nc.tensor.matmul(
                        pt[:],
                        wt[:, 2 * p : 2 * p + 2, :],
                        rhs,
                        start=(p == 0),
                        stop=(p == 4),
                        perf_mode=mybir.MatmulPerfMode.DoubleRow,
                    )
                # PSUM -> SBUF eviction with per-channel Alpha scale on DVE,
                # skipping the 2 garbage columns per row.
                nc.vector.tensor_scalar_mul(
                    o_sb[:, ch * FREE : (ch + 1) * FREE].rearrange(
                        "c (a b) -> c a b", b=W
                    ),
                    pt[:].rearrange("c (a b) -> c a b", b=PW)[:, :, 0:W],
                    alpha_sb[:, 0:1],
                )
            nc.sync.dma_start(o_ap[i].rearrange("c h w -> c (h w)"), o_sb[:])

    nc.finalize()
    return nc


_CACHE = {}


def _get_nc(rv):
    key = rv.tobytes()
    if key not in _CACHE:
        _CACHE[key] = build_kernel(np.asarray(rv, np.float32).reshape(-1))
    return _CACHE[key]


def _run(inputs, trace=False):
    x = np.ascontiguousarray(np.asarray(inputs["x"], np.float32))
    M = np.ascontiguousarray(np.asarray(inputs["M"], np.float32))
    Z = np.ascontiguousarray(np.asarray(inputs["Z"], np.float32))
    Alpha = np.ascontiguousarray(np.asarray(inputs["Alpha"], np.float32))
    rv = np.ascontiguousarray(np.asarray(inputs["rv"], np.float32))
    nc = _get_nc(rv)
    in_maps = [
        {"x": x[c * BPC : (c + 1) * BPC], "M": M, "Z": Z, "Alpha": Alpha, "rv": rv}
        for c in range(N_CORES)
    ]
    res = run_bass_kernel_spmd(nc, in_maps, list(range(N_CORES)), trace=trace)
    out = np.concatenate([res.results[c]["out"] for c in range(N_CORES)], axis=0)
    return out, res


def kernel(**inputs):
    out, _ = _run(inputs, trace=False)
    return out


def kernel_traced(**inputs):
    out, res = _run(inputs, trace=True)
    return out, res


# revision 19
# speedup vs baseline: 1.1353x; 1.0120x over previous
"""BinarizeConv2dSDP kernel for Trainium2 (8 NeuronCores, data-parallel over batch).

out = conv2d(sign(x), sign(M + sum_k rv[k] * Z[k]), stride 1, pad 1) * Alpha

Key simplification: the reference normalizes (M, Z) by rsqrt(M^2 + sum Z^2 / SCALE)
before forming w = rv@Z + M, but that factor is strictly positive and applied
multiplicatively to the whole expression, so sign(w) is unaffected.  The binary
weights are just sign(M + sum_k rv[k] Z[k]).

Strategy per core (8 images each):
  - weight gen on DVE: w = M + sum rv_k Z_k in [oc, ic*9] layout, sign -> bf16,
    then 9 PE transposes give lhsT tiles [ic, oc] per kernel tap.
  - conv as 9 accumulated matmuls per output chunk: contraction over ic=128
    partitions, PSUM accumulates the 3x3 taps over a 58-wide zero-padded
    activation image (sign(x) in bf16).
  - Alpha scaling on DVE during PSUM->SBUF eviction, one output DMA per image.
"""

import numpy as np
from contextlib import ExitStack

import concourse.bass as bass
import concourse.mybir as mybir
import concourse.tile as tile
from concourse.bacc import Bacc
from concourse.bass_utils import run_bass_kernel_spmd
from concourse.tile_rust import add_dep_helper

N_CORES = 8
B, C, H, W = 64, 128, 56, 56
BPC = B // N_CORES  # images per core
KS, K = 3, 5
PH, PW = H + 2, W + 2  # zero-padded image
CHUNK_ROWS = 8
N_CHUNKS = H // CHUNK_ROWS
FREE = CHUNK_ROWS * W  # matmul free dim (448 <= 512 psum bank)
F32 = mybir.dt.float32
BF16 = mybir.dt.bfloat16
F8 = mybir.dt.float8e4


def build_kernel(rv_vals):
    """Build the single-core Bass module (SPMD: same program on all 8 cores).

    rv_vals: the 5 rv scalars, baked as immediates into the weight-gen ops.
    """
    # Bacc (not plain Bass): its compile() pass pipeline legalizes sync waits
    # (TRN2 allows at most 1 embedded wait per engine instruction; excess waits
    # are split into InstEventSemaphore via generate_event_semaphores).
    nc = Bacc()
    x_p = nc.declare_dram_parameter("x", [BPC, C, H, W], F32, isOutput=False)
    m_p = nc.declare_dram_parameter("M", [C, C, KS, KS], F32, isOutput=False)
    z_p = nc.declare_dram_parameter("Z", [K, C, C, KS, KS], F32, isOutput=False)
    a_p = nc.declare_dram_parameter("Alpha", [C, 1, 1], F32, isOutput=False)
    rv_p = nc.declare_dram_parameter("rv", [1, K], F32, isOutput=False)
    out_p = nc.declare_dram_parameter("out", [BPC, C, H, W], F32, isOutput=True)

    from concourse.masks import make_identity

    NW = C * KS * KS  # 1152 weight elements per out-channel row

    with tile.TileContext(nc) as tc, ExitStack() as ctx:
        const = ctx.enter_context(tc.tile_pool(name="const", bufs=1))
        wg = ctx.enter_context(tc.tile_pool(name="wg", bufs=1))
        zpool = ctx.enter_context(tc.tile_pool(name="zpool", bufs=1))
        xin = ctx.enter_context(tc.tile_pool(name="xin", bufs=4))
        pad = ctx.enter_context(tc.tile_pool(name="pad", bufs=3))
        opool = ctx.enter_context(tc.tile_pool(name="opool", bufs=2))
        ps_t = ctx.enter_context(tc.tile_pool(name="ps_t", bufs=2, space="PSUM"))
        ps_c = ctx.enter_context(tc.tile_pool(name="ps_c", bufs=6, space="PSUM"))

        # ---- constants ----
        identity = const.tile([C, C], BF16)
        make_identity(nc, identity)
        alpha_sb = const.tile([C, 1], F32)
        nc.sync.dma_start(alpha_sb[:], a_p[:].rearrange("c a b -> c (a b)"))
        # rv is consumed via baked immediates; still touch the input so the
        # parameter is used (run path requires binding it anyway).
        rv_sb = const.tile([1, K], F32)
        nc.sync.dma_start(rv_sb[:], rv_p[:])

        # ---- weight generation: w = M + sum_k rv_k Z_k, in [oc, ic*3*3] ----
        m_sb = wg.tile([C, NW], F32)
        nc.sync.dma_start(m_sb[:], m_p[:].rearrange("o i a b -> o (i a b)"))
        z_sbs = []
        z_dma_insts = []
        for k in range(K):
            z_sb = zpool.tile([C, NW], F32, tag=f"z{k}")
            d = nc.sync.dma_start(z_sb[:], z_p[k].rearrange("o i a b -> o (i a b)"))
            z_sbs.append(z_sb)
            z_dma_insts.append(d)
        # Absorb the DMA-completion waits with cheap copies: the TensorScalarPtr
        # (S2S2D2_STT) instruction format below cannot carry sync waits, so its
        # DVE dependencies must already be settled in the engine's vector clock.
        scratch = wg.tile([C, 8], F32)
        for k in range(K):
            nc.vector.tensor_copy(scratch[:, k : k + 1], z_sbs[k][:, 0:1])
        nc.vector.tensor_copy(scratch[0:1, K : K + 1], rv_sb[0:1, 0:1])
        nc.vector.tensor_copy(scratch[:, K + 1 : K + 2], m_sb[:, 0:1])
        w_sb = wg.tile([C, NW], F32)
        nc.vector.tensor_copy(w_sb[:], m_sb[:])
        for k in range(K):
            # w = (z_k * rv_k) + w
            nc.vector.scalar_tensor_tensor(
                w_sb[:],
                z_sbs[k][:],
                float(rv_vals[k]),
                w_sb[:],
                mybir.AluOpType.mult,
                mybir.AluOpType.add,
            )
        bw_sb = wg.tile([C, NW], BF16)
        nc.scalar.sign(bw_sb[:], w_sb[:])

        # transpose each tap's [oc, ic] into lhsT [ic, oc]; store as fp8e4
        # (+-1 is exact) with a 10th all-zero tap so the 9 taps pair up into
        # 5 DoubleRow matmuls (2 fp8 weights per PE cell = 2x throughput).
        wt = const.tile([C, KS * KS + 1, C], F8)
        nc.vector.memset(wt[:, KS * KS, :], 0.0)
        bw_r = bw_sb[:].rearrange("o (i j) -> o i j", j=KS * KS)
        for j in range(KS * KS):
            tp = ps_t.tile([C, C], BF16)
            nc.tensor.transpose(tp[:], bw_r[:, :, j], identity[:])
            nc.vector.tensor_copy(wt[:, j, :], tp[:])

        # ---- conv main loop ----
        x_ap = x_p[:]
        o_ap = out_p[:]

        def tap_off(r0, j):
            # flat offset of (out-row r0, tap j)'s top-left read in the padded image
            if j == KS * KS:  # zero tap: alias tap 8's window (weights are 0)
                j = KS * KS - 1
            return (r0 + j // KS) * PW + (j % KS)

        # Matmul free dim spans whole padded rows (8*58=464 contiguous, so the
        # DoubleRow moving AP stays 3D [C, 2, 464]); each row's last 2 output
        # columns are garbage that the eviction skips.  +2 trailing elements
        # keep the last tap's 464-wide read in bounds.
        FREE_R = CHUNK_ROWS * PW  # 464 <= 512 psum bank
        for i in range(BPC):
            x_sb = xin.tile([C, H * W], F32)
            xd = nc.sync.dma_start(x_sb[:], x_ap[i].rearrange("c h w -> c (h w)"))
            if i < 3:
                # Don't let early image loads steal HBM bandwidth from the
                # weight-gen DMAs: the first conv matmul can't start until the
                # weights are generated, so those 3.5MB must land first.
                add_dep_helper(
                    xd.ins, z_dma_insts[-1].ins, reason="weight DMAs drain first"
                )
            ba = pad.tile([C, PH * PW + 2], F8)
            nc.gpsimd.memset(ba[:], 0.0)
            ba_r = ba[:, 0 : PH * PW].rearrange("c (h w) -> c h w", w=PW)
            nc.scalar.sign(
                ba_r[:, 1 : H + 1, 1 : W + 1],
                x_sb[:].rearrange("c (h w) -> c h w", w=W),
            )
            o_sb = opool.tile([C, H * W], F32)
            for ch in range(N_CHUNKS):
                pt = ps_c.tile([C, FREE_R], F32)
                r0 = ch * CHUNK_ROWS
                for p in range(5):
                    o0 = tap_off(r0, 2 * p)
                    o1 = tap_off(r0, 2 * p + 1)
                    rhs = bass.AP(
                        ba[:].tensor,
                        o0,
                        [[PH * PW + 2, C], [o1 - o0, 2], [1, FREE_R]],
                    )
                    nc.tensor.matmul(
                        pt[:],
                        wt[:, 2 * p : 2 * p + 2, :],
                        rhs,
                        start=(p == 0),
                        stop=(p == 4),
                        perf_mode=mybir.MatmulPerfMode.DoubleRow,
                    )
                # PSUM -> SBUF eviction with per-channel Alpha scale on DVE,
                # skipping the 2 garbage columns per row.
                nc.vector.tensor_scalar_mul(
                    o_sb[:, ch * FREE : (ch + 1) * FREE].rearrange(
                        "c (a b) -> c a b", b=W
                    ),
                    pt[:].rearrange("c (a b) -> c a b", b=PW)[:, :, 0:W],
                    alpha_sb[:, 0:1],
                )
            nc.sync.dma_start(o_ap[i].rearrange("c h w -> c (h w)"), o_sb[:])

    nc.finalize()
    return nc


_CACHE = {}


def _get_nc(rv):
    key = rv.tobytes()
    if key not in _CACHE:
        _CACHE[key] = build_kernel(np.asarray(rv, np.float32).reshape(-1))
    return _CACHE[key]


def _run(inputs, trace=False):
    x = np.ascontiguousarray(np.asarray(inputs["x"], np.float32))
    M = np.ascontiguousarray(np.asarray(inputs["M"], np.float32))
    Z = np.ascontiguousarray(np.asarray(inputs["Z"], np.float32))
    Alpha = np.ascontiguousarray(np.asarray(inputs["Alpha"], np.float32))
    rv = np.ascontiguousarray(np.asarray(inputs["rv"], np.float32))
    nc = _get_nc(rv)
    in_maps = [
        {"x": x[c * BPC : (c + 1) * BPC], "M": M, "Z": Z, "Alpha": Alpha, "rv": rv}
        for c in range(N_CORES)
    ]
    res = run_bass_kernel_spmd(nc, in_maps, list(range(N_CORES)), trace=trace)
    out = np.concatenate([res.results[c]["out"] for c in range(N_CORES)], axis=0)
    return out, res


def kernel(**inputs):
    out, _ = _run(inputs, trace=False)
    return out


def kernel_traced(**inputs):
    out, res = _run(inputs, trace=True)
    return out, res
